# revision 24
# baseline (speedup 1.0000x reference)
"""Trainium2 Bass kernel for the ADMM total-variation solver (nn_ADMM).

Math: the reference iterates x <- resid @ inv(AtA + g*DtD + a*I) 50 times.
AtA is rank-9 (A is [9, 4096]) and C := g*DtD + a*I is a symmetric
tridiagonal circulant, so by Woodbury

    B^-1 = C^-1 - U S^-1 U^T,   U = C^-1 A^T,  S = I9 + A U

where C^-1 is a circulant whose kernel decays like 0.5^|d|.  The dense
4096x4096 matvec therefore becomes a banded (radius-32) circular
convolution plus a rank-9 correction - tiny enough to keep everything
resident in SBUF on a single NeuronCore with no HBM traffic inside the
iteration loop.  All 8 cores run the same program (SPMD, no collectives);
core 0's output is returned.

Device layout: vectors live as [128, 34] tiles with halo columns,
flat index i = k + 128*c stored at tile[:, c+1]; col 0 mirrors col 32
and col 33 mirrors col 1.  A banded circulant operator is then 2-3
matmuls: a [128,128] main stationary on cols 1:33 plus corner blocks
(padded to K=128) on the column-shifted views - the halo provides the
circular wrap for free.

Per iteration (scaled states Eb = eta, Tb = tau, Ub = g*u, Wb = a*w):
PE computes only the banded convolutions into PSUM (plus bf16 filler
matmuls that keep the HAM clock ramp at 2.4 GHz); cross-tensor adds and
the soft-threshold (max(z-lam,0) + min(z+lam,0)) run on DVE; alpha*x
and the j=1 relu run on ACT; state prep and halo copies on GpSimd.
The rank-9 term q = U^T v is a DVE multiply+reduce, and a single
all-ones [128,128] matmul performs the partition reduction AND the
broadcast of q in one shot.
"""

import numpy as np

N = 4096
P = 128          # partitions
CCOL = 32        # payload columns; i = k + 128*c at tile col c+1
HCOL = CCOL + 2  # halo tile width
RB = 32          # band radius
RBLK = 64        # corner block active rows
R9 = 9           # Woodbury rank
GAMMA = 10.0
ALPHA = 5.0
LAM = 1e-4
NIT = 50
NCORES = 8

# column offsets inside the constant blob [128, BLOB_COLS]
OFF_OPA1_M = 0       # gamma*(S+ - I) main
OFF_OPB_M = 128      # gamma*(S- - I) main
OFF_G_M = 256        # C^-1 banded main
OFF_AI = 384         # alpha*I
OFF_ONESF = 512      # all-ones (partition reduce + broadcast)
OFF_OPA1_BL = 640    # corner blocks, K padded to 128
OFF_OPB_BH = 768
OFF_G_BL = 896
OFF_G_BH = 1024
OFF_USTACK2 = 1152   # U, m-major: [k, m*32+c]
OFF_W2STACK = 1440   # W2, c-major: [k, c*9+m]
OFF_C0 = 1728        # B^-1 bA tile
OFF_X0 = 1760        # x0 tile WITH halo columns [128, 34]
OFF_MLAM = 1794      # column of -LAM (activation bias AP)
OFF_I = 1824         # identity (c0 accumulate pair in the E group)
BLOB_COLS = 1952


def _banded_mats(h):
    """lhsT pieces for kernel h (dict d -> coef), all [*, 128] columns.
    main: within-column;  BL: reads column c-1 (rows 64..127 active);
    BH: reads column c+1 (rows 0..63 active).  Corner arrays are
    returned [128, 128] with inactive rows zero (K padded to 128 so all
    matmuls share tile_position (0, 0))."""
    B0 = np.zeros((P, P), np.float64)
    BL = np.zeros((P, P), np.float64)
    BH = np.zeros((P, P), np.float64)
    for k in range(P):
        for m in range(P):
            d = k - m
            if d in h:
                B0[k, m] = h[d]
    for a in range(RBLK):
        for m in range(P):
            d = (a + P - RBLK) - P - m      # BL active row k' = 64 + a
            if d in h:
                BL[a + P - RBLK, m] = h[d]
            d = a + P - m                   # BH active row k' = a
            if d in h:
                BH[a, m] = h[d]
    return B0, BL, BH


def _tile(vec):
    """[4096] -> [128, 32], i = k + 128*c."""
    return np.ascontiguousarray(vec.reshape(CCOL, P).T)


def host_constants(target, A, x0):
    """All f64 precompute; returns the [128, BLOB_COLS] f32 device blob."""
    A64 = np.asarray(A, np.float64)
    w = ALPHA + 2 * GAMMA * (1 - np.cos(2 * np.pi * np.arange(N // 2 + 1) / N))

    def C_inv(z):
        return np.fft.irfft(np.fft.rfft(z, axis=-1) / w, n=N, axis=-1)

    U = C_inv(A64).T                              # [N, 9]
    S = np.eye(R9) + A64 @ U
    W2 = U @ np.linalg.inv(S)                     # [N, 9]
    g = np.fft.irfft(1.0 / w, n=N)                # kernel of C^-1
    b = A64 @ np.asarray(target, np.float64)
    bA = b @ A64
    c0 = C_inv(bA) - W2 @ (U.T @ bA)              # B^-1 @ bA

    blob = np.zeros((P, BLOB_COLS), np.float64)

    def put(off, arr):
        blob[:, off:off + arr.shape[1]] = arr

    mA1 = _banded_mats({-1: GAMMA, 0: -GAMMA})    # gamma*(S+ - I)
    mB = _banded_mats({1: GAMMA, 0: -GAMMA})      # gamma*(S- - I)
    mG = _banded_mats({d: g[d % N] for d in range(-RB, RB + 1)})
    put(OFF_OPA1_M, mA1[0]); put(OFF_OPA1_BL, mA1[1])
    put(OFF_OPB_M, mB[0]); put(OFF_OPB_BH, mB[2])
    put(OFF_G_M, mG[0]); put(OFF_G_BL, mG[1]); put(OFF_G_BH, mG[2])
    put(OFF_AI, ALPHA * np.eye(P))
    put(OFF_ONESF, np.ones((P, P)))

    # Ustack2[k, m*32+c] = U[k + 128c, m]  (m-major, for q = U^T v)
    put(OFF_USTACK2, U.reshape(CCOL, P, R9).transpose(1, 2, 0).reshape(P, R9 * CCOL))
    # W2stack[k, c*9+m] = W2[k + 128c, m]  (c-major, for corr = W2 q)
    put(OFF_W2STACK, W2.reshape(CCOL, P, R9).transpose(1, 0, 2).reshape(P, CCOL * R9))
    put(OFF_C0, _tile(c0))
    put(OFF_I, np.eye(P))

    x0t = _tile(np.asarray(x0, np.float64))
    blob[:, OFF_X0 + 1:OFF_X0 + 1 + CCOL] = x0t
    blob[:, OFF_X0] = x0t[:, CCOL - 1]            # halo: col0 = col32
    blob[:, OFF_X0 + CCOL + 1] = x0t[:, 0]        # halo: col33 = col1
    blob[:, OFF_MLAM] = -LAM
    return np.ascontiguousarray(blob.astype(np.float32))


def build_nc():
    """Build and compile the Bacc graph (one core's program)."""
    from concourse import bacc, mybir, tile

    f32 = mybir.dt.float32
    Relu = mybir.ActivationFunctionType.Relu
    Alu = mybir.AluOpType
    nc = bacc.Bacc(target_bir_lowering=False)

    blob_ext = nc.declare_dram_parameter("blob", [P, BLOB_COLS], f32, isOutput=False)
    out_ext = nc.declare_dram_parameter("out", [P, CCOL], f32, isOutput=True)

    with tile.TileContext(nc) as tc:
        with (
            tc.tile_pool(name="const", bufs=1) as cpool,
            tc.tile_pool(name="work", bufs=2) as wpool,
            tc.tile_pool(name="psum", bufs=1, space="PSUM") as ppool,
        ):
            cb = cpool.tile([P, BLOB_COLS], f32, tag="blob")
            nc.sync.dma_start(cb[:, :], blob_ext[:, :])

            def cs(off, width):
                return cb[:, off:off + width]

            A1_M, A1_BL = cs(OFF_OPA1_M, P), cs(OFF_OPA1_BL, P)
            B_M, B_BH = cs(OFF_OPB_M, P), cs(OFF_OPB_BH, P)
            G_M, G_BL, G_BH = cs(OFF_G_M, P), cs(OFF_G_BL, P), cs(OFF_G_BH, P)
            Copy = mybir.ActivationFunctionType.Copy
            onesf = cs(OFF_ONESF, P)
            U2_3d = cs(OFF_USTACK2, R9 * CCOL).rearrange("k (m c) -> k m c", c=CCOL)
            W2_3d = cs(OFF_W2STACK, CCOL * R9).rearrange("k (c m) -> k c m", m=R9)
            c0_t = cs(OFF_C0, CCOL)
            I_t = cs(OFF_I, P)
            mlam = cb[:, OFF_MLAM:OFF_MLAM + 1]

            def emit_bank(mms):
                n = len(mms)
                for i, (o, l, r) in enumerate(mms):
                    nc.tensor.matmul(o, l, r, start=(i == 0), stop=(i == n - 1))

            def halo_fix(xh):
                """mirror payload edge columns into the halo columns"""
                nc.gpsimd.tensor_copy(xh[:, 0:1], xh[:, CCOL:CCOL + 1])
                nc.gpsimd.tensor_copy(xh[:, HCOL - 1:HCOL], xh[:, 1:2])

            # state (python refs); X is a halo tile view [128, 34]
            Xh = cs(OFF_X0, HCOL)
            E_sb = T_sb = U_sb = W_sb = None

            NWARM = 8
            bf16 = mybir.dt.bfloat16
            warm_w = cs(OFF_ONESF, P).bitcast(bf16)[:, 0:P]
            warm_x = cs(OFF_ONESF, P).bitcast(bf16)[:, 0:2 * P]

            for j in range(1, NIT + 1):
                first = (j == 1)
                second = (j == 2)

                # keep the PE clock warm with filler bf16 matmuls
                if NWARM:
                    scratch = ppool.tile([P, 2 * P], f32, tag="scratch")
                    for wi in range(NWARM):
                        nc.tensor.matmul(scratch[:, :], warm_w, warm_x,
                                         start=True, stop=True)

                bankA = ppool.tile([P, CCOL], f32, tag="A")
                bankE = ppool.tile([P, CCOL], f32, tag="E")
                bankR = ppool.tile([P, R9], f32, tag="R")
                if not first:
                    bankB = ppool.tile([P, CCOL], f32, tag="B")

                xm, xl, xr = Xh[:, 1:CCOL + 1], Xh[:, 0:CCOL], Xh[:, 2:HCOL]

                # early combos from previous state (GpSimd: off critical path)
                if not first:
                    tAB = wpool.tile([P, CCOL], f32, tag="tAB")
                    tTW = wpool.tile([P, CCOL], f32, tag="tTW")
                    if second:
                        nc.gpsimd.tensor_scalar_mul(tAB[:, :], U_sb[:, :], -1.0)
                        nc.gpsimd.tensor_scalar_mul(tTW[:, :], W_sb[:, :], -1.0)
                    else:
                        nc.gpsimd.tensor_sub(tAB[:, :], E_sb[:, :], U_sb[:, :])
                        nc.gpsimd.tensor_sub(tTW[:, :], T_sb[:, :], W_sb[:, :])

                # PE: banded mains only; wrap corners are single elements
                # for A1/B and get applied as GpSimd row fixes below
                if not first:
                    emit_bank([(bankB[:, :], B_M, xm), (bankB[:, :], B_BH, xr)])
                    emit_bank([(bankA[:, :], A1_M, xm), (bankA[:, :], A1_BL, xl)])
                else:
                    emit_bank([(bankA[:, :], A1_M, xm), (bankA[:, :], A1_BL, xl)])

                r1 = wpool.tile([P, CCOL], f32, tag="r1")
                m2 = wpool.tile([P, CCOL], f32, tag="m2")
                Un = wpool.tile([P, CCOL], f32, tag="Un")
                Wn = wpool.tile([P, CCOL], f32, tag="Wn")
                vh = wpool.tile([P, HCOL], f32, tag="vh")
                vm, vl, vr = vh[:, 1:CCOL + 1], vh[:, 0:CCOL], vh[:, 2:HCOL]

                if first:
                    At_src = bankA[:, :]
                    En = Tn = None
                    # Wb1 = relu(alpha * x0)
                    nc.scalar.activation(Wn[:, :], xm, Relu, bias=0.0, scale=ALPHA)
                else:
                    Cx = wpool.tile([P, CCOL], f32, tag="Cx")
                    En = wpool.tile([P, CCOL], f32, tag="En")
                    Tn = wpool.tile([P, CCOL], f32, tag="Tn")
                    At = wpool.tile([P, CCOL], f32, tag="At")
                    Dt = wpool.tile([P, CCOL], f32, tag="Dt")
                    nc.scalar.activation(Cx[:, :], xm,
                                         mybir.ActivationFunctionType.Copy,
                                         bias=0.0, scale=ALPHA)
                    nc.vector.tensor_add(En[:, :], bankB[:, :], tAB[:, :])
                    nc.vector.tensor_add(At[:, :], bankA[:, :], En[:, :])
                    nc.vector.tensor_add(Tn[:, :], Cx[:, :], tTW[:, :])
                    nc.vector.tensor_add(Dt[:, :], Cx[:, :], Tn[:, :])
                    At_src = At[:, :]
                # soft threshold on DVE: soft(z) = max(z-lam,0) + min(z+lam,0)
                nc.vector.tensor_scalar(r1[:, :], At_src, -LAM, 0.0,
                                        Alu.add, Alu.max)
                nc.vector.tensor_scalar(m2[:, :], At_src, LAM, 0.0,
                                        Alu.add, Alu.min)
                nc.vector.tensor_add(Un[:, :], r1[:, :], m2[:, :])
                if first:
                    nc.vector.tensor_add(vm, Un[:, :], Wn[:, :])
                else:
                    P1 = wpool.tile([P, CCOL], f32, tag="P1")
                    P2n = wpool.tile([P, CCOL], f32, tag="P2n")
                    # P1 = relu(Dt) + Un in one op; Wn state off-path (GpSimd)
                    nc.vector.scalar_tensor_tensor(P1[:, :], Dt[:, :], 0.0,
                                                   Un[:, :], Alu.max, Alu.add)
                    nc.gpsimd.tensor_scalar_max(Wn[:, :], Dt[:, :], 0.0)
                    # P2n = -(En + Tn)
                    nc.vector.scalar_tensor_tensor(P2n[:, :], En[:, :], -1.0,
                                                   Tn[:, :], Alu.mult, Alu.subtract)
                    nc.vector.tensor_add(vm, P1[:, :], P2n[:, :])
                halo_fix(vh)

                # y = G v + c0 (PE).  Emitted in two pieces with the
                # rank-9 reduce/broadcast matmul (bankR) in between, so the
                # Z2 chain starts while the G corner matmuls still run.

                # rank-9: q = U^T v via DVE; all-ones matmul reduces over
                # partitions AND broadcasts q to [128, 9]
                Z1 = wpool.tile([P, R9 * CCOL], f32, tag="Z1")
                Z1r = wpool.tile([P, R9], f32, tag="Z1r")
                z1_3d = Z1[:, :].rearrange("k (m c) -> k m c", c=CCOL)
                vb = vm.unsqueeze(1).broadcast_to([P, R9, CCOL])
                nc.vector.tensor_mul(z1_3d, U2_3d, vb)
                nc.vector.tensor_reduce(Z1r[:, :], z1_3d, axis=mybir.AxisListType.X,
                                        op=Alu.add)
                nc.tensor.matmul(bankE[:, :], I_t, c0_t, start=True, stop=False)
                nc.tensor.matmul(bankE[:, :], G_M, vm, start=False, stop=False)
                emit_bank([(bankR[:, :], onesf, Z1r[:, :])])
                nc.tensor.matmul(bankE[:, :], G_BL, vl, start=False, stop=False)
                nc.tensor.matmul(bankE[:, :], G_BH, vr, start=False, stop=True)

                Z2 = wpool.tile([P, CCOL * R9], f32, tag="Z2")
                corr = wpool.tile([P, CCOL], f32, tag="corr")
                z2_3d = Z2[:, :].rearrange("k (c m) -> k c m", m=R9)
                rb = bankR[:, :].unsqueeze(1).broadcast_to([P, CCOL, R9])
                nc.vector.tensor_mul(z2_3d, W2_3d, rb)
                nc.vector.tensor_reduce(corr[:, :], z2_3d, axis=mybir.AxisListType.X,
                                        op=Alu.add)

                # x' = bankE - corr   (c0 accumulated into bankE on PE)
                Xn = wpool.tile([P, HCOL], f32, tag="Xh")
                nc.vector.tensor_sub(Xn[:, 1:CCOL + 1], bankE[:, :], corr[:, :])
                halo_fix(Xn)

                Xh, E_sb, T_sb, U_sb, W_sb = Xn, En, Tn, Un, Wn

            nc.sync.dma_start(out_ext[:, :], Xh[:, 1:CCOL + 1])

    nc.compile()
    return nc


def kernel(**inputs):
    from concourse.bass_utils import run_bass_kernel_spmd

    target = np.asarray(inputs["target"], np.float32)
    A = np.asarray(inputs["A"], np.float32)
    x0 = np.asarray(inputs["x0"], np.float32)

    blob = host_constants(target, A, x0)
    nc = build_nc()
    in_maps = [{"blob": blob} for _ in range(NCORES)]
    res = run_bass_kernel_spmd(nc, in_maps, core_ids=list(range(NCORES)))
    out_tile = np.asarray(res.results[0]["out"], np.float32)
    return np.ascontiguousarray(out_tile.T.reshape(-1))


# revision 25
# speedup vs baseline: 1.0507x; 1.0507x over previous
"""Trainium2 Bass kernel for the ADMM total-variation solver (nn_ADMM).

Math: the reference iterates x <- resid @ inv(AtA + g*DtD + a*I) 50 times.
AtA is rank-9 (A is [9, 4096]) and C := g*DtD + a*I is a symmetric
tridiagonal circulant, so by Woodbury

    B^-1 = C^-1 - U S^-1 U^T,   U = C^-1 A^T,  S = I9 + A U

where C^-1 is a circulant whose kernel decays like 0.5^|d|.  The dense
4096x4096 matvec therefore becomes a banded (radius-32) circular
convolution plus a rank-9 correction - tiny enough to keep everything
resident in SBUF on a single NeuronCore with no HBM traffic inside the
iteration loop.  All 8 cores run the same program (SPMD, no collectives);
core 0's output is returned.

Device layout: vectors live as [128, 34] tiles with halo columns,
flat index i = k + 128*c stored at tile[:, c+1]; col 0 mirrors col 32
and col 33 mirrors col 1.  A banded circulant operator is then 2-3
matmuls: a [128,128] main stationary on cols 1:33 plus corner blocks
(padded to K=128) on the column-shifted views - the halo provides the
circular wrap for free.

Per iteration (scaled states Eb = eta, Tb = tau, Ub = g*u, Wb = a*w):
PE computes only the banded convolutions into PSUM (plus bf16 filler
matmuls that keep the HAM clock ramp at 2.4 GHz); cross-tensor adds and
the soft-threshold (max(z-lam,0) + min(z+lam,0)) run on DVE; alpha*x
and the j=1 relu run on ACT; state prep and halo copies on GpSimd.
The rank-9 term q = U^T v is a DVE multiply+reduce, and a single
all-ones [128,128] matmul performs the partition reduction AND the
broadcast of q in one shot.
"""

import numpy as np

N = 4096
P = 128          # partitions
CCOL = 32        # payload columns; i = k + 128*c at tile col c+1
HCOL = CCOL + 2  # halo tile width
RB = 32          # band radius
RBLK = 64        # corner block active rows
R9 = 9           # Woodbury rank
GAMMA = 10.0
ALPHA = 5.0
LAM = 1e-4
NIT = 50
NCORES = 8

# column offsets inside the constant blob [128, BLOB_COLS]
OFF_OPA1_M = 0       # gamma*(S+ - I) main
OFF_OPB_M = 128      # gamma*(S- - I) main
OFF_G_M = 256        # C^-1 banded main
OFF_AI = 384         # alpha*I
OFF_ONESF = 512      # all-ones (partition reduce + broadcast)
OFF_OPA1_BL = 640    # corner blocks, K padded to 128
OFF_OPB_BH = 768
OFF_G_BL = 896
OFF_G_BH = 1024
OFF_USTACK2 = 1152   # U, m-major: [k, m*32+c]
OFF_W2STACK = 1440   # W2, c-major: [k, c*9+m]
OFF_C0 = 1728        # B^-1 bA tile
OFF_X0 = 1760        # x0 tile WITH halo columns [128, 34]
OFF_MLAM = 1794      # column of -LAM (activation bias AP)
OFF_I = 1824         # identity (c0 accumulate pair in the E group)
BLOB_COLS = 1952


def _banded_mats(h):
    """lhsT pieces for kernel h (dict d -> coef), all [*, 128] columns.
    main: within-column;  BL: reads column c-1 (rows 64..127 active);
    BH: reads column c+1 (rows 0..63 active).  Corner arrays are
    returned [128, 128] with inactive rows zero (K padded to 128 so all
    matmuls share tile_position (0, 0))."""
    B0 = np.zeros((P, P), np.float64)
    BL = np.zeros((P, P), np.float64)
    BH = np.zeros((P, P), np.float64)
    for k in range(P):
        for m in range(P):
            d = k - m
            if d in h:
                B0[k, m] = h[d]
    for a in range(RBLK):
        for m in range(P):
            d = (a + P - RBLK) - P - m      # BL active row k' = 64 + a
            if d in h:
                BL[a + P - RBLK, m] = h[d]
            d = a + P - m                   # BH active row k' = a
            if d in h:
                BH[a, m] = h[d]
    return B0, BL, BH


def _tile(vec):
    """[4096] -> [128, 32], i = k + 128*c."""
    return np.ascontiguousarray(vec.reshape(CCOL, P).T)


def host_constants(target, A, x0):
    """All f64 precompute; returns the [128, BLOB_COLS] f32 device blob."""
    A64 = np.asarray(A, np.float64)
    w = ALPHA + 2 * GAMMA * (1 - np.cos(2 * np.pi * np.arange(N // 2 + 1) / N))

    def C_inv(z):
        return np.fft.irfft(np.fft.rfft(z, axis=-1) / w, n=N, axis=-1)

    U = C_inv(A64).T                              # [N, 9]
    S = np.eye(R9) + A64 @ U
    W2 = U @ np.linalg.inv(S)                     # [N, 9]
    g = np.fft.irfft(1.0 / w, n=N)                # kernel of C^-1
    b = A64 @ np.asarray(target, np.float64)
    bA = b @ A64
    c0 = C_inv(bA) - W2 @ (U.T @ bA)              # B^-1 @ bA

    blob = np.zeros((P, BLOB_COLS), np.float64)

    def put(off, arr):
        blob[:, off:off + arr.shape[1]] = arr

    mA1 = _banded_mats({-1: GAMMA, 0: -GAMMA})    # gamma*(S+ - I)
    mB = _banded_mats({1: GAMMA, 0: -GAMMA})      # gamma*(S- - I)
    mG = _banded_mats({d: g[d % N] for d in range(-RB, RB + 1)})
    put(OFF_OPA1_M, mA1[0]); put(OFF_OPA1_BL, mA1[1])
    put(OFF_OPB_M, mB[0]); put(OFF_OPB_BH, mB[2])
    put(OFF_G_M, mG[0]); put(OFF_G_BL, mG[1]); put(OFF_G_BH, mG[2])
    put(OFF_AI, ALPHA * np.eye(P))
    put(OFF_ONESF, np.ones((P, P)))

    # Ustack2[k, m*32+c] = U[k + 128c, m]  (m-major, for q = U^T v)
    put(OFF_USTACK2, U.reshape(CCOL, P, R9).transpose(1, 2, 0).reshape(P, R9 * CCOL))
    # W2stack[k, c*9+m] = W2[k + 128c, m]  (c-major, for corr = W2 q)
    put(OFF_W2STACK, W2.reshape(CCOL, P, R9).transpose(1, 0, 2).reshape(P, CCOL * R9))
    put(OFF_C0, _tile(c0))
    put(OFF_I, np.eye(P))

    x0t = _tile(np.asarray(x0, np.float64))
    blob[:, OFF_X0 + 1:OFF_X0 + 1 + CCOL] = x0t
    blob[:, OFF_X0] = x0t[:, CCOL - 1]            # halo: col0 = col32
    blob[:, OFF_X0 + CCOL + 1] = x0t[:, 0]        # halo: col33 = col1
    blob[:, OFF_MLAM] = -LAM
    return np.ascontiguousarray(blob.astype(np.float32))


def build_nc():
    """Build and compile the Bacc graph (one core's program)."""
    from concourse import bacc, mybir, tile

    f32 = mybir.dt.float32
    Relu = mybir.ActivationFunctionType.Relu
    Alu = mybir.AluOpType
    nc = bacc.Bacc(target_bir_lowering=False)

    blob_ext = nc.declare_dram_parameter("blob", [P, BLOB_COLS], f32, isOutput=False)
    out_ext = nc.declare_dram_parameter("out", [P, CCOL], f32, isOutput=True)

    with tile.TileContext(nc) as tc:
        with (
            tc.tile_pool(name="const", bufs=1) as cpool,
            tc.tile_pool(name="work", bufs=2) as wpool,
            tc.tile_pool(name="psum", bufs=1, space="PSUM") as ppool,
        ):
            cb = cpool.tile([P, BLOB_COLS], f32, tag="blob")
            nc.sync.dma_start(cb[:, :], blob_ext[:, :])

            def cs(off, width):
                return cb[:, off:off + width]

            A1_M, A1_BL = cs(OFF_OPA1_M, P), cs(OFF_OPA1_BL, P)
            B_M, B_BH = cs(OFF_OPB_M, P), cs(OFF_OPB_BH, P)
            G_M, G_BL, G_BH = cs(OFF_G_M, P), cs(OFF_G_BL, P), cs(OFF_G_BH, P)
            Copy = mybir.ActivationFunctionType.Copy
            onesf = cs(OFF_ONESF, P)
            U2_3d = cs(OFF_USTACK2, R9 * CCOL).rearrange("k (m c) -> k m c", c=CCOL)
            W2_3d = cs(OFF_W2STACK, CCOL * R9).rearrange("k (c m) -> k c m", m=R9)
            c0_t = cs(OFF_C0, CCOL)
            I_t = cs(OFF_I, P)
            mlam = cb[:, OFF_MLAM:OFF_MLAM + 1]

            def emit_bank(mms):
                n = len(mms)
                for i, (o, l, r) in enumerate(mms):
                    nc.tensor.matmul(o, l, r, start=(i == 0), stop=(i == n - 1))

            def halo_fix(xh):
                """mirror payload edge columns into the halo columns"""
                nc.gpsimd.tensor_copy(xh[:, 0:1], xh[:, CCOL:CCOL + 1])
                nc.gpsimd.tensor_copy(xh[:, HCOL - 1:HCOL], xh[:, 1:2])

            # state (python refs); X is a halo tile view [128, 34]
            Xh = cs(OFF_X0, HCOL)
            E_sb = T_sb = U_sb = W_sb = None

            NWARM = 8
            bf16 = mybir.dt.bfloat16
            warm_w = cs(OFF_ONESF, P).bitcast(bf16)[:, 0:P]
            warm_x = cs(OFF_ONESF, P).bitcast(bf16)[:, 0:2 * P]

            for j in range(1, NIT + 1):
                first = (j == 1)
                second = (j == 2)

                # keep the PE clock warm with filler bf16 matmuls
                if NWARM:
                    scratch = ppool.tile([P, 2 * P], f32, tag="scratch")
                    for wi in range(NWARM):
                        nc.tensor.matmul(scratch[:, :], warm_w, warm_x,
                                         start=True, stop=True)

                bankA = ppool.tile([P, CCOL], f32, tag="A")
                bankE = ppool.tile([P, CCOL], f32, tag="E")
                bankR = ppool.tile([P, R9], f32, tag="R")
                if not first:
                    bankB = ppool.tile([P, CCOL], f32, tag="B")

                xm, xl, xr = Xh[:, 1:CCOL + 1], Xh[:, 0:CCOL], Xh[:, 2:HCOL]

                # early combos from previous state (GpSimd: off critical path)
                if not first:
                    tAB = wpool.tile([P, CCOL], f32, tag="tAB")
                    tTW = wpool.tile([P, CCOL], f32, tag="tTW")
                    if second:
                        nc.gpsimd.tensor_scalar_mul(tAB[:, :], U_sb[:, :], -1.0)
                        nc.gpsimd.tensor_scalar_mul(tTW[:, :], W_sb[:, :], -1.0)
                    else:
                        nc.gpsimd.tensor_sub(tAB[:, :], E_sb[:, :], U_sb[:, :])
                        nc.gpsimd.tensor_sub(tTW[:, :], T_sb[:, :], W_sb[:, :])

                # PE: banded mains only; wrap corners are single elements
                # for A1/B and get applied as GpSimd row fixes below
                if not first:
                    nc.tensor.matmul(bankB[:, :], B_M, xm, start=True, stop=False)
                    nc.tensor.matmul(bankA[:, :], A1_M, xm, start=True, stop=False)
                    nc.tensor.matmul(bankB[:, :], B_BH, xr, start=False, stop=True)
                    nc.tensor.matmul(bankA[:, :], A1_BL, xl, start=False, stop=True)
                else:
                    emit_bank([(bankA[:, :], A1_M, xm), (bankA[:, :], A1_BL, xl)])

                r1 = wpool.tile([P, CCOL], f32, tag="r1")
                Un = wpool.tile([P, CCOL], f32, tag="Un")
                Wn = wpool.tile([P, CCOL], f32, tag="Wn")
                vh = wpool.tile([P, HCOL], f32, tag="vh")
                vm, vl, vr = vh[:, 1:CCOL + 1], vh[:, 0:CCOL], vh[:, 2:HCOL]

                if first:
                    At_src = bankA[:, :]
                    En = Tn = None
                    # Wb1 = relu(alpha * x0)
                    nc.scalar.activation(Wn[:, :], xm, Relu, bias=0.0, scale=ALPHA)
                else:
                    Cx = wpool.tile([P, CCOL], f32, tag="Cx")
                    En = wpool.tile([P, CCOL], f32, tag="En")
                    Tn = wpool.tile([P, CCOL], f32, tag="Tn")
                    At = wpool.tile([P, CCOL], f32, tag="At")
                    Dt = wpool.tile([P, CCOL], f32, tag="Dt")
                    nc.scalar.activation(Cx[:, :], xm,
                                         mybir.ActivationFunctionType.Copy,
                                         bias=0.0, scale=ALPHA)
                    nc.vector.tensor_add(En[:, :], bankB[:, :], tAB[:, :])
                    nc.vector.tensor_add(At[:, :], bankA[:, :], En[:, :])
                    nc.vector.tensor_add(Tn[:, :], Cx[:, :], tTW[:, :])
                    nc.vector.tensor_add(Dt[:, :], Cx[:, :], Tn[:, :])
                    At_src = At[:, :]
                # soft threshold on DVE in 2 ops: soft(z) = z - clip(z,-lam,lam)
                nc.vector.tensor_scalar(r1[:, :], At_src, -LAM, LAM,
                                        Alu.max, Alu.min)
                nc.vector.tensor_sub(Un[:, :], At_src, r1[:, :])
                if first:
                    nc.vector.tensor_add(vm, Un[:, :], Wn[:, :])
                else:
                    P1 = wpool.tile([P, CCOL], f32, tag="P1")
                    P2n = wpool.tile([P, CCOL], f32, tag="P2n")
                    # P1 = relu(Dt) + Un in one op; Wn state off-path (GpSimd)
                    nc.vector.scalar_tensor_tensor(P1[:, :], Dt[:, :], 0.0,
                                                   Un[:, :], Alu.max, Alu.add)
                    nc.gpsimd.tensor_scalar_max(Wn[:, :], Dt[:, :], 0.0)
                    # P2n = -(En + Tn)
                    nc.vector.scalar_tensor_tensor(P2n[:, :], En[:, :], -1.0,
                                                   Tn[:, :], Alu.mult, Alu.subtract)
                    nc.vector.tensor_add(vm, P1[:, :], P2n[:, :])
                halo_fix(vh)

                # y = G v + c0 (PE).  Emitted in two pieces with the
                # rank-9 reduce/broadcast matmul (bankR) in between, so the
                # Z2 chain starts while the G corner matmuls still run.

                # rank-9: q = U^T v via DVE; all-ones matmul reduces over
                # partitions AND broadcasts q to [128, 9]
                Z1 = wpool.tile([P, R9 * CCOL], f32, tag="Z1")
                Z1r = wpool.tile([P, R9], f32, tag="Z1r")
                z1_3d = Z1[:, :].rearrange("k (m c) -> k m c", c=CCOL)
                vb = vm.unsqueeze(1).broadcast_to([P, R9, CCOL])
                nc.vector.tensor_mul(z1_3d, U2_3d, vb)
                nc.vector.tensor_reduce(Z1r[:, :], z1_3d, axis=mybir.AxisListType.X,
                                        op=Alu.add)
                nc.tensor.matmul(bankE[:, :], I_t, c0_t, start=True, stop=False)
                nc.tensor.matmul(bankE[:, :], G_M, vm, start=False, stop=False)
                emit_bank([(bankR[:, :], onesf, Z1r[:, :])])
                nc.tensor.matmul(bankE[:, :], G_BL, vl, start=False, stop=False)
                nc.tensor.matmul(bankE[:, :], G_BH, vr, start=False, stop=True)

                Z2 = wpool.tile([P, CCOL * R9], f32, tag="Z2")
                corr = wpool.tile([P, CCOL], f32, tag="corr")
                z2_3d = Z2[:, :].rearrange("k (c m) -> k c m", m=R9)
                rb = bankR[:, :].unsqueeze(1).broadcast_to([P, CCOL, R9])
                nc.vector.tensor_mul(z2_3d, W2_3d, rb)
                nc.vector.tensor_reduce(corr[:, :], z2_3d, axis=mybir.AxisListType.X,
                                        op=Alu.add)

                # x' = bankE - corr   (c0 accumulated into bankE on PE)
                Xn = wpool.tile([P, HCOL], f32, tag="Xh")
                nc.vector.tensor_sub(Xn[:, 1:CCOL + 1], bankE[:, :], corr[:, :])
                halo_fix(Xn)

                Xh, E_sb, T_sb, U_sb, W_sb = Xn, En, Tn, Un, Wn

            nc.sync.dma_start(out_ext[:, :], Xh[:, 1:CCOL + 1])

    nc.compile()
    return nc


def kernel(**inputs):
    from concourse.bass_utils import run_bass_kernel_spmd

    target = np.asarray(inputs["target"], np.float32)
    A = np.asarray(inputs["A"], np.float32)
    x0 = np.asarray(inputs["x0"], np.float32)

    blob = host_constants(target, A, x0)
    nc = build_nc()
    in_maps = [{"blob": blob} for _ in range(NCORES)]
    res = run_bass_kernel_spmd(nc, in_maps, core_ids=list(range(NCORES)))
    out_tile = np.asarray(res.results[0]["out"], np.float32)
    return np.ascontiguousarray(out_tile.T.reshape(-1))


# revision 27
# speedup vs baseline: 1.0527x; 1.0018x over previous
"""Trainium2 Bass kernel for the ADMM total-variation solver (nn_ADMM).

Math: the reference iterates x <- resid @ inv(AtA + g*DtD + a*I) 50 times.
AtA is rank-9 (A is [9, 4096]) and C := g*DtD + a*I is a symmetric
tridiagonal circulant, so by Woodbury

    B^-1 = C^-1 - U S^-1 U^T,   U = C^-1 A^T,  S = I9 + A U

where C^-1 is a circulant whose kernel decays like 0.5^|d|.  The dense
4096x4096 matvec therefore becomes a banded (radius-32) circular
convolution plus a rank-9 correction - tiny enough to keep everything
resident in SBUF on a single NeuronCore with no HBM traffic inside the
iteration loop.  All 8 cores run the same program (SPMD, no collectives);
core 0's output is returned.

Device layout: vectors live as [128, 34] tiles with halo columns,
flat index i = k + 128*c stored at tile[:, c+1]; col 0 mirrors col 32
and col 33 mirrors col 1.  A banded circulant operator is then 2-3
matmuls: a [128,128] main stationary on cols 1:33 plus corner blocks
(padded to K=128) on the column-shifted views - the halo provides the
circular wrap for free.

Per iteration (scaled states Eb = eta, Tb = tau, Ub = g*u, Wb = a*w):
PE computes only the banded convolutions into PSUM (plus bf16 filler
matmuls that keep the HAM clock ramp at 2.4 GHz); cross-tensor adds and
the soft-threshold (z - clip(z,-lam,lam), two DVE ops) run on DVE; alpha*x
and the j=1 relu run on ACT; state prep and halo copies on GpSimd.
The rank-9 term q = U^T v is a DVE multiply+reduce, and a single
all-ones [128,128] matmul performs the partition reduction AND the
broadcast of q in one shot.
"""

import numpy as np

N = 4096
P = 128          # partitions
CCOL = 32        # payload columns; i = k + 128*c at tile col c+1
HCOL = CCOL + 2  # halo tile width
RB = 32          # band radius
RBLK = 64        # corner block active rows
R9 = 9           # Woodbury rank
GAMMA = 10.0
ALPHA = 5.0
LAM = 1e-4
NIT = 50
NCORES = 8

# column offsets inside the constant blob [128, BLOB_COLS]
OFF_OPA1_M = 0       # gamma*(S+ - I) main
OFF_OPB_M = 128      # gamma*(S- - I) main
OFF_G_M = 256        # C^-1 banded main
OFF_AI = 384         # alpha*I
OFF_ONESF = 512      # all-ones (partition reduce + broadcast)
OFF_OPA1_BL = 640    # corner blocks, K padded to 128
OFF_OPB_BH = 768
OFF_G_BL = 896
OFF_G_BH = 1024
OFF_USTACK2 = 1152   # U, m-major: [k, m*32+c]
OFF_W2STACK = 1440   # W2, c-major: [k, c*9+m]
OFF_C0 = 1728        # B^-1 bA tile
OFF_X0 = 1760        # x0 tile WITH halo columns [128, 34]
OFF_MLAM = 1794      # column of -LAM (activation bias AP)
OFF_I = 1824         # identity (c0 accumulate pair in the E group)
BLOB_COLS = 1952


def _banded_mats(h):
    """lhsT pieces for kernel h (dict d -> coef), all [*, 128] columns.
    main: within-column;  BL: reads column c-1 (rows 64..127 active);
    BH: reads column c+1 (rows 0..63 active).  Corner arrays are
    returned [128, 128] with inactive rows zero (K padded to 128 so all
    matmuls share tile_position (0, 0))."""
    B0 = np.zeros((P, P), np.float64)
    BL = np.zeros((P, P), np.float64)
    BH = np.zeros((P, P), np.float64)
    for k in range(P):
        for m in range(P):
            d = k - m
            if d in h:
                B0[k, m] = h[d]
    for a in range(RBLK):
        for m in range(P):
            d = (a + P - RBLK) - P - m      # BL active row k' = 64 + a
            if d in h:
                BL[a + P - RBLK, m] = h[d]
            d = a + P - m                   # BH active row k' = a
            if d in h:
                BH[a, m] = h[d]
    return B0, BL, BH


def _tile(vec):
    """[4096] -> [128, 32], i = k + 128*c."""
    return np.ascontiguousarray(vec.reshape(CCOL, P).T)


def host_constants(target, A, x0):
    """All f64 precompute; returns the [128, BLOB_COLS] f32 device blob."""
    A64 = np.asarray(A, np.float64)
    w = ALPHA + 2 * GAMMA * (1 - np.cos(2 * np.pi * np.arange(N // 2 + 1) / N))

    def C_inv(z):
        return np.fft.irfft(np.fft.rfft(z, axis=-1) / w, n=N, axis=-1)

    U = C_inv(A64).T                              # [N, 9]
    S = np.eye(R9) + A64 @ U
    W2 = U @ np.linalg.inv(S)                     # [N, 9]
    g = np.fft.irfft(1.0 / w, n=N)                # kernel of C^-1
    b = A64 @ np.asarray(target, np.float64)
    bA = b @ A64
    c0 = C_inv(bA) - W2 @ (U.T @ bA)              # B^-1 @ bA

    blob = np.zeros((P, BLOB_COLS), np.float64)

    def put(off, arr):
        blob[:, off:off + arr.shape[1]] = arr

    mA1 = _banded_mats({-1: GAMMA, 0: -GAMMA})    # gamma*(S+ - I)
    mB = _banded_mats({1: GAMMA, 0: -GAMMA})      # gamma*(S- - I)
    mG = _banded_mats({d: g[d % N] for d in range(-RB, RB + 1)})
    put(OFF_OPA1_M, mA1[0]); put(OFF_OPA1_BL, mA1[1])
    put(OFF_OPB_M, mB[0]); put(OFF_OPB_BH, mB[2])
    put(OFF_G_M, mG[0]); put(OFF_G_BL, mG[1]); put(OFF_G_BH, mG[2])
    put(OFF_AI, ALPHA * np.eye(P))
    put(OFF_ONESF, np.ones((P, P)))

    # Ustack2[k, m*32+c] = U[k + 128c, m]  (m-major, for q = U^T v)
    put(OFF_USTACK2, U.reshape(CCOL, P, R9).transpose(1, 2, 0).reshape(P, R9 * CCOL))
    # W2stack[k, c*9+m] = W2[k + 128c, m]  (c-major, for corr = W2 q)
    put(OFF_W2STACK, W2.reshape(CCOL, P, R9).transpose(1, 0, 2).reshape(P, CCOL * R9))
    put(OFF_C0, _tile(c0))
    put(OFF_I, np.eye(P))

    x0t = _tile(np.asarray(x0, np.float64))
    blob[:, OFF_X0 + 1:OFF_X0 + 1 + CCOL] = x0t
    blob[:, OFF_X0] = x0t[:, CCOL - 1]            # halo: col0 = col32
    blob[:, OFF_X0 + CCOL + 1] = x0t[:, 0]        # halo: col33 = col1
    blob[:, OFF_MLAM] = -LAM
    return np.ascontiguousarray(blob.astype(np.float32))


def build_nc():
    """Build and compile the Bacc graph (one core's program)."""
    from concourse import bacc, mybir, tile

    f32 = mybir.dt.float32
    Relu = mybir.ActivationFunctionType.Relu
    Alu = mybir.AluOpType
    nc = bacc.Bacc(target_bir_lowering=False)

    blob_ext = nc.declare_dram_parameter("blob", [P, BLOB_COLS], f32, isOutput=False)
    out_ext = nc.declare_dram_parameter("out", [P, CCOL], f32, isOutput=True)

    with tile.TileContext(nc) as tc:
        with (
            tc.tile_pool(name="const", bufs=1) as cpool,
            tc.tile_pool(name="work", bufs=3) as wpool,
            tc.tile_pool(name="psum", bufs=1, space="PSUM") as ppool,
        ):
            cb = cpool.tile([P, BLOB_COLS], f32, tag="blob")
            nc.sync.dma_start(cb[:, :], blob_ext[:, :])

            def cs(off, width):
                return cb[:, off:off + width]

            A1_M, A1_BL = cs(OFF_OPA1_M, P), cs(OFF_OPA1_BL, P)
            B_M, B_BH = cs(OFF_OPB_M, P), cs(OFF_OPB_BH, P)
            G_M, G_BL, G_BH = cs(OFF_G_M, P), cs(OFF_G_BL, P), cs(OFF_G_BH, P)
            Copy = mybir.ActivationFunctionType.Copy
            onesf = cs(OFF_ONESF, P)
            U2_3d = cs(OFF_USTACK2, R9 * CCOL).rearrange("k (m c) -> k m c", c=CCOL)
            W2_3d = cs(OFF_W2STACK, CCOL * R9).rearrange("k (c m) -> k c m", m=R9)
            c0_t = cs(OFF_C0, CCOL)
            I_t = cs(OFF_I, P)
            mlam = cb[:, OFF_MLAM:OFF_MLAM + 1]

            def emit_bank(mms):
                n = len(mms)
                for i, (o, l, r) in enumerate(mms):
                    nc.tensor.matmul(o, l, r, start=(i == 0), stop=(i == n - 1))

            def halo_fix(xh):
                """mirror payload edge columns into the halo columns"""
                nc.gpsimd.tensor_copy(xh[:, 0:1], xh[:, CCOL:CCOL + 1])
                nc.gpsimd.tensor_copy(xh[:, HCOL - 1:HCOL], xh[:, 1:2])

            # state (python refs); X is a halo tile view [128, 34]
            Xh = cs(OFF_X0, HCOL)
            E_sb = T_sb = U_sb = W_sb = None

            NWARM = 8
            bf16 = mybir.dt.bfloat16
            warm_w = cs(OFF_ONESF, P).bitcast(bf16)[:, 0:P]
            warm_x = cs(OFF_ONESF, P).bitcast(bf16)[:, 0:2 * P]

            for j in range(1, NIT + 1):
                first = (j == 1)
                second = (j == 2)

                # keep the PE clock warm with filler bf16 matmuls
                if NWARM:
                    scratch = ppool.tile([P, 2 * P], f32, tag="scratch")
                    for wi in range(NWARM):
                        nc.tensor.matmul(scratch[:, :], warm_w, warm_x,
                                         start=True, stop=True)

                bankA = ppool.tile([P, CCOL], f32, tag="A")
                bankE = ppool.tile([P, CCOL], f32, tag="E")
                bankR = ppool.tile([P, R9], f32, tag="R")
                if not first:
                    bankB = ppool.tile([P, CCOL], f32, tag="B")

                xm, xl, xr = Xh[:, 1:CCOL + 1], Xh[:, 0:CCOL], Xh[:, 2:HCOL]

                # early combos from previous state (GpSimd: off critical path)
                if not first:
                    tAB = wpool.tile([P, CCOL], f32, tag="tAB")
                    tTW = wpool.tile([P, CCOL], f32, tag="tTW")
                    if second:
                        nc.gpsimd.tensor_scalar_mul(tAB[:, :], U_sb[:, :], -1.0)
                        nc.gpsimd.tensor_scalar_mul(tTW[:, :], W_sb[:, :], -1.0)
                    else:
                        nc.gpsimd.tensor_sub(tAB[:, :], E_sb[:, :], U_sb[:, :])
                        nc.gpsimd.tensor_sub(tTW[:, :], T_sb[:, :], W_sb[:, :])

                # PE: banded mains only; wrap corners are single elements
                # for A1/B and get applied as GpSimd row fixes below
                if not first:
                    nc.tensor.matmul(bankB[:, :], B_M, xm, start=True, stop=False)
                    nc.tensor.matmul(bankA[:, :], A1_M, xm, start=True, stop=False)
                    nc.tensor.matmul(bankB[:, :], B_BH, xr, start=False, stop=True)
                    nc.tensor.matmul(bankA[:, :], A1_BL, xl, start=False, stop=True)
                else:
                    emit_bank([(bankA[:, :], A1_M, xm), (bankA[:, :], A1_BL, xl)])

                r1 = wpool.tile([P, CCOL], f32, tag="r1")
                Un = wpool.tile([P, CCOL], f32, tag="Un")
                Wn = wpool.tile([P, CCOL], f32, tag="Wn")
                vh = wpool.tile([P, HCOL], f32, tag="vh")
                vm, vl, vr = vh[:, 1:CCOL + 1], vh[:, 0:CCOL], vh[:, 2:HCOL]

                if first:
                    At_src = bankA[:, :]
                    En = Tn = None
                    # Wb1 = relu(alpha * x0)
                    nc.scalar.activation(Wn[:, :], xm, Relu, bias=0.0, scale=ALPHA)
                else:
                    Cx = wpool.tile([P, CCOL], f32, tag="Cx")
                    En = wpool.tile([P, CCOL], f32, tag="En")
                    Tn = wpool.tile([P, CCOL], f32, tag="Tn")
                    At = wpool.tile([P, CCOL], f32, tag="At")
                    Dt = wpool.tile([P, CCOL], f32, tag="Dt")
                    nc.scalar.activation(Cx[:, :], xm,
                                         mybir.ActivationFunctionType.Copy,
                                         bias=0.0, scale=ALPHA)
                    nc.vector.tensor_add(En[:, :], bankB[:, :], tAB[:, :])
                    nc.vector.tensor_add(At[:, :], bankA[:, :], En[:, :])
                    nc.vector.tensor_add(Tn[:, :], Cx[:, :], tTW[:, :])
                    nc.vector.tensor_add(Dt[:, :], Cx[:, :], Tn[:, :])
                    At_src = At[:, :]
                # soft threshold on DVE in 2 ops: soft(z) = z - clip(z,-lam,lam)
                nc.vector.tensor_scalar(r1[:, :], At_src, -LAM, LAM,
                                        Alu.max, Alu.min)
                nc.vector.tensor_sub(Un[:, :], At_src, r1[:, :])
                if first:
                    nc.vector.tensor_add(vm, Un[:, :], Wn[:, :])
                else:
                    P1 = wpool.tile([P, CCOL], f32, tag="P1")
                    P2n = wpool.tile([P, CCOL], f32, tag="P2n")
                    # P1 = relu(Dt) + Un in one op; Wn state off-path (GpSimd)
                    nc.vector.scalar_tensor_tensor(P1[:, :], Dt[:, :], 0.0,
                                                   Un[:, :], Alu.max, Alu.add)
                    nc.gpsimd.tensor_scalar_max(Wn[:, :], Dt[:, :], 0.0)
                    # P2n = -(En + Tn)
                    nc.vector.scalar_tensor_tensor(P2n[:, :], En[:, :], -1.0,
                                                   Tn[:, :], Alu.mult, Alu.subtract)
                    nc.vector.tensor_add(vm, P1[:, :], P2n[:, :])
                halo_fix(vh)

                # y = G v + c0 (PE).  Emitted in two pieces with the
                # rank-9 reduce/broadcast matmul (bankR) in between, so the
                # Z2 chain starts while the G corner matmuls still run.

                # rank-9: q = U^T v via DVE; all-ones matmul reduces over
                # partitions AND broadcasts q to [128, 9]
                Z1 = wpool.tile([P, R9 * CCOL], f32, tag="Z1")
                Z1r = wpool.tile([P, R9], f32, tag="Z1r")
                z1_3d = Z1[:, :].rearrange("k (m c) -> k m c", c=CCOL)
                vb = vm.unsqueeze(1).broadcast_to([P, R9, CCOL])
                nc.vector.tensor_mul(z1_3d, U2_3d, vb)
                nc.vector.tensor_reduce(Z1r[:, :], z1_3d, axis=mybir.AxisListType.X,
                                        op=Alu.add)
                nc.tensor.matmul(bankE[:, :], I_t, c0_t, start=True, stop=False)
                nc.tensor.matmul(bankE[:, :], G_M, vm, start=False, stop=False)
                emit_bank([(bankR[:, :], onesf, Z1r[:, :])])
                nc.tensor.matmul(bankE[:, :], G_BL, vl, start=False, stop=False)
                nc.tensor.matmul(bankE[:, :], G_BH, vr, start=False, stop=True)

                Z2 = wpool.tile([P, CCOL * R9], f32, tag="Z2")
                corr = wpool.tile([P, CCOL], f32, tag="corr")
                z2_3d = Z2[:, :].rearrange("k (c m) -> k c m", m=R9)
                rb = bankR[:, :].unsqueeze(1).broadcast_to([P, CCOL, R9])
                nc.vector.tensor_mul(z2_3d, W2_3d, rb)
                nc.vector.tensor_reduce(corr[:, :], z2_3d, axis=mybir.AxisListType.X,
                                        op=Alu.add)

                # x' = bankE - corr   (c0 accumulated into bankE on PE)
                Xn = wpool.tile([P, HCOL], f32, tag="Xh")
                nc.vector.tensor_sub(Xn[:, 1:CCOL + 1], bankE[:, :], corr[:, :])
                halo_fix(Xn)

                Xh, E_sb, T_sb, U_sb, W_sb = Xn, En, Tn, Un, Wn

            nc.sync.dma_start(out_ext[:, :], Xh[:, 1:CCOL + 1])

    nc.compile()
    return nc


def kernel(**inputs):
    from concourse.bass_utils import run_bass_kernel_spmd

    target = np.asarray(inputs["target"], np.float32)
    A = np.asarray(inputs["A"], np.float32)
    x0 = np.asarray(inputs["x0"], np.float32)

    blob = host_constants(target, A, x0)
    nc = build_nc()
    in_maps = [{"blob": blob} for _ in range(NCORES)]
    res = run_bass_kernel_spmd(nc, in_maps, core_ids=list(range(NCORES)))
    out_tile = np.asarray(res.results[0]["out"], np.float32)
    return np.ascontiguousarray(out_tile.T.reshape(-1))


# revision 30
# speedup vs baseline: 1.0560x; 1.0031x over previous
"""Trainium2 Bass kernel for the ADMM total-variation solver (nn_ADMM).

Math: the reference iterates x <- resid @ inv(AtA + g*DtD + a*I) 50 times.
AtA is rank-9 (A is [9, 4096]) and C := g*DtD + a*I is a symmetric
tridiagonal circulant, so by Woodbury

    B^-1 = C^-1 - U S^-1 U^T,   U = C^-1 A^T,  S = I9 + A U

where C^-1 is a circulant whose kernel decays like 0.5^|d|.  The dense
4096x4096 matvec therefore becomes a banded (radius-32) circular
convolution plus a rank-9 correction - tiny enough to keep everything
resident in SBUF on a single NeuronCore with no HBM traffic inside the
iteration loop.  All 8 cores run the same program (SPMD, no collectives);
core 0's output is returned.

Device layout: vectors live as [128, 34] tiles with halo columns,
flat index i = k + 128*c stored at tile[:, c+1]; col 0 mirrors col 32
and col 33 mirrors col 1.  A banded circulant operator is then 2-3
matmuls: a [128,128] main stationary on cols 1:33 plus corner blocks
(padded to K=128) on the column-shifted views - the halo provides the
circular wrap for free.

Per iteration (scaled states Eb = eta, Tb = tau, Ub = g*u, Wb = a*w):
PE computes only the banded convolutions into PSUM (plus bf16 filler
matmuls that keep the HAM clock ramp at 2.4 GHz); cross-tensor adds and
the soft-threshold (z - clip(z,-lam,lam), two DVE ops) run on DVE; alpha*x
and the j=1 relu run on ACT; state prep and halo copies on GpSimd.
The rank-9 term q = U^T v is a DVE multiply+reduce, and a single
all-ones [128,128] matmul performs the partition reduction AND the
broadcast of q in one shot.
"""

import numpy as np

N = 4096
P = 128          # partitions
CCOL = 32        # payload columns; i = k + 128*c at tile col c+1
HCOL = CCOL + 2  # halo tile width
RB = 32          # band radius
RBLK = 64        # corner block active rows
R9 = 9           # Woodbury rank
GAMMA = 10.0
ALPHA = 5.0
LAM = 1e-4
NIT = 50
NCORES = 8

# column offsets inside the constant blob [128, BLOB_COLS]
OFF_OPA1_M = 0       # gamma*(S+ - I) main
OFF_OPB_M = 128      # gamma*(S- - I) main
OFF_G_M = 256        # C^-1 banded main
OFF_AI = 384         # alpha*I
OFF_ONESF = 512      # all-ones (partition reduce + broadcast)
OFF_OPA1_BL = 640    # corner blocks, K padded to 128
OFF_OPB_BH = 768
OFF_G_BL = 896
OFF_G_BH = 1024
OFF_USTACK2 = 1152   # U, m-major: [k, m*32+c]
OFF_W2STACK = 1440   # W2, c-major: [k, c*9+m]
OFF_C0 = 1728        # B^-1 bA tile
OFF_X0 = 1760        # x0 tile WITH halo columns [128, 34]
OFF_MLAM = 1794      # column of -LAM (activation bias AP)
OFF_I = 1824         # identity (c0 accumulate pair in the E group)
BLOB_COLS = 1952


def _banded_mats(h):
    """lhsT pieces for kernel h (dict d -> coef), all [*, 128] columns.
    main: within-column;  BL: reads column c-1 (rows 64..127 active);
    BH: reads column c+1 (rows 0..63 active).  Corner arrays are
    returned [128, 128] with inactive rows zero (K padded to 128 so all
    matmuls share tile_position (0, 0))."""
    B0 = np.zeros((P, P), np.float64)
    BL = np.zeros((P, P), np.float64)
    BH = np.zeros((P, P), np.float64)
    for k in range(P):
        for m in range(P):
            d = k - m
            if d in h:
                B0[k, m] = h[d]
    for a in range(RBLK):
        for m in range(P):
            d = (a + P - RBLK) - P - m      # BL active row k' = 64 + a
            if d in h:
                BL[a + P - RBLK, m] = h[d]
            d = a + P - m                   # BH active row k' = a
            if d in h:
                BH[a, m] = h[d]
    return B0, BL, BH


def _tile(vec):
    """[4096] -> [128, 32], i = k + 128*c."""
    return np.ascontiguousarray(vec.reshape(CCOL, P).T)


def host_constants(target, A, x0):
    """All f64 precompute; returns the [128, BLOB_COLS] f32 device blob."""
    A64 = np.asarray(A, np.float64)
    w = ALPHA + 2 * GAMMA * (1 - np.cos(2 * np.pi * np.arange(N // 2 + 1) / N))

    def C_inv(z):
        return np.fft.irfft(np.fft.rfft(z, axis=-1) / w, n=N, axis=-1)

    U = C_inv(A64).T                              # [N, 9]
    S = np.eye(R9) + A64 @ U
    W2 = U @ np.linalg.inv(S)                     # [N, 9]
    g = np.fft.irfft(1.0 / w, n=N)                # kernel of C^-1
    b = A64 @ np.asarray(target, np.float64)
    bA = b @ A64
    c0 = C_inv(bA) - W2 @ (U.T @ bA)              # B^-1 @ bA

    blob = np.zeros((P, BLOB_COLS), np.float64)

    def put(off, arr):
        blob[:, off:off + arr.shape[1]] = arr

    mA1 = _banded_mats({-1: GAMMA, 0: -GAMMA})    # gamma*(S+ - I)
    mB = _banded_mats({1: GAMMA, 0: -GAMMA})      # gamma*(S- - I)
    mG = _banded_mats({d: g[d % N] for d in range(-RB, RB + 1)})
    put(OFF_OPA1_M, mA1[0]); put(OFF_OPA1_BL, mA1[1])
    put(OFF_OPB_M, mB[0]); put(OFF_OPB_BH, mB[2])
    put(OFF_G_M, mG[0]); put(OFF_G_BL, mG[1]); put(OFF_G_BH, mG[2])
    put(OFF_AI, ALPHA * np.eye(P))
    put(OFF_ONESF, np.ones((P, P)))

    # Ustack2[k, m*32+c] = U[k + 128c, m]  (m-major, for q = U^T v)
    put(OFF_USTACK2, U.reshape(CCOL, P, R9).transpose(1, 2, 0).reshape(P, R9 * CCOL))
    # W2stack[k, c*9+m] = W2[k + 128c, m]  (c-major, for corr = W2 q)
    put(OFF_W2STACK, W2.reshape(CCOL, P, R9).transpose(1, 0, 2).reshape(P, CCOL * R9))
    put(OFF_C0, _tile(c0))
    put(OFF_I, np.eye(P))

    x0t = _tile(np.asarray(x0, np.float64))
    blob[:, OFF_X0 + 1:OFF_X0 + 1 + CCOL] = x0t
    blob[:, OFF_X0] = x0t[:, CCOL - 1]            # halo: col0 = col32
    blob[:, OFF_X0 + CCOL + 1] = x0t[:, 0]        # halo: col33 = col1
    blob[:, OFF_MLAM] = -LAM
    return np.ascontiguousarray(blob.astype(np.float32))


def build_nc():
    """Build and compile the Bacc graph (one core's program)."""
    from concourse import bacc, mybir, tile

    f32 = mybir.dt.float32
    Relu = mybir.ActivationFunctionType.Relu
    Alu = mybir.AluOpType
    nc = bacc.Bacc(target_bir_lowering=False)

    blob_ext = nc.declare_dram_parameter("blob", [P, BLOB_COLS], f32, isOutput=False)
    out_ext = nc.declare_dram_parameter("out", [P, CCOL], f32, isOutput=True)

    with tile.TileContext(nc) as tc:
        with (
            tc.tile_pool(name="const", bufs=1) as cpool,
            tc.tile_pool(name="work", bufs=3) as wpool,
            tc.tile_pool(name="psum", bufs=1, space="PSUM") as ppool,
        ):
            cb = cpool.tile([P, BLOB_COLS], f32, tag="blob")
            nc.sync.dma_start(cb[:, :], blob_ext[:, :])

            def cs(off, width):
                return cb[:, off:off + width]

            A1_M, A1_BL = cs(OFF_OPA1_M, P), cs(OFF_OPA1_BL, P)
            B_M, B_BH = cs(OFF_OPB_M, P), cs(OFF_OPB_BH, P)
            G_M, G_BL, G_BH = cs(OFF_G_M, P), cs(OFF_G_BL, P), cs(OFF_G_BH, P)
            Copy = mybir.ActivationFunctionType.Copy
            onesf = cs(OFF_ONESF, P)
            U2_3d = cs(OFF_USTACK2, R9 * CCOL).rearrange("k (m c) -> k m c", c=CCOL)
            W2_3d = cs(OFF_W2STACK, CCOL * R9).rearrange("k (c m) -> k c m", m=R9)
            c0_t = cs(OFF_C0, CCOL)
            I_t = cs(OFF_I, P)
            mlam = cb[:, OFF_MLAM:OFF_MLAM + 1]

            def emit_bank(mms):
                n = len(mms)
                for i, (o, l, r) in enumerate(mms):
                    nc.tensor.matmul(o, l, r, start=(i == 0), stop=(i == n - 1))

            def halo_fix(xh):
                """mirror payload edge columns into the halo columns"""
                nc.gpsimd.tensor_copy(xh[:, 0:1], xh[:, CCOL:CCOL + 1])
                nc.gpsimd.tensor_copy(xh[:, HCOL - 1:HCOL], xh[:, 1:2])

            # state (python refs); X is a halo tile view [128, 34]
            Xh = cs(OFF_X0, HCOL)
            E_sb = T_sb = U_sb = W_sb = None

            NWARM = 8
            bf16 = mybir.dt.bfloat16
            warm_w = cs(OFF_ONESF, P).bitcast(bf16)[:, 0:P]
            warm_x = cs(OFF_ONESF, P).bitcast(bf16)[:, 0:2 * P]

            for j in range(1, NIT + 1):
                first = (j == 1)
                second = (j == 2)

                # keep the PE clock warm with filler bf16 matmuls
                if NWARM:
                    scratch = ppool.tile([P, 2 * P], f32, tag="scratch")
                    for wi in range(NWARM):
                        nc.tensor.matmul(scratch[:, :], warm_w, warm_x,
                                         start=True, stop=True)

                bankA = ppool.tile([P, CCOL], f32, tag="A")
                bankE = ppool.tile([P, CCOL], f32, tag="E")
                bankR = ppool.tile([P, R9], f32, tag="R")
                if not first:
                    bankB = ppool.tile([P, CCOL], f32, tag="B")

                xm, xl, xr = Xh[:, 1:CCOL + 1], Xh[:, 0:CCOL], Xh[:, 2:HCOL]

                # early combos from previous state (GpSimd: off critical path)
                if not first:
                    tAB = wpool.tile([P, CCOL], f32, tag="tAB")
                    tTW = wpool.tile([P, CCOL], f32, tag="tTW")
                    if second:
                        nc.gpsimd.tensor_scalar_mul(tAB[:, :], U_sb[:, :], -1.0)
                        nc.gpsimd.tensor_scalar_mul(tTW[:, :], W_sb[:, :], -1.0)
                    else:
                        nc.gpsimd.tensor_sub(tAB[:, :], E_sb[:, :], U_sb[:, :])
                        nc.gpsimd.tensor_sub(tTW[:, :], T_sb[:, :], W_sb[:, :])

                # PE: banded mains only; wrap corners are single elements
                # for A1/B and get applied as GpSimd row fixes below
                if not first:
                    nc.tensor.matmul(bankB[:, :], B_M, xm, start=True, stop=False)
                    nc.tensor.matmul(bankA[:, :], A1_M, xm, start=True, stop=False)
                    nc.tensor.matmul(bankB[:, :], B_BH, xr, start=False, stop=True)
                    nc.tensor.matmul(bankA[:, :], A1_BL, xl, start=False, stop=True)
                else:
                    emit_bank([(bankA[:, :], A1_M, xm), (bankA[:, :], A1_BL, xl)])

                r1 = wpool.tile([P, CCOL], f32, tag="r1")
                Un = wpool.tile([P, CCOL], f32, tag="Un")
                Wn = wpool.tile([P, CCOL], f32, tag="Wn")
                vh = wpool.tile([P, HCOL], f32, tag="vh")
                vm, vl, vr = vh[:, 1:CCOL + 1], vh[:, 0:CCOL], vh[:, 2:HCOL]

                if first:
                    At_src = bankA[:, :]
                    En = Tn = None
                    # Wb1 = relu(alpha * x0)
                    nc.scalar.activation(Wn[:, :], xm, Relu, bias=0.0, scale=ALPHA)
                else:
                    Cx = wpool.tile([P, CCOL], f32, tag="Cx")
                    En = wpool.tile([P, CCOL], f32, tag="En")
                    Tn = wpool.tile([P, CCOL], f32, tag="Tn")
                    At = wpool.tile([P, CCOL], f32, tag="At")
                    Dt = wpool.tile([P, CCOL], f32, tag="Dt")
                    nc.scalar.activation(Cx[:, :], xm,
                                         mybir.ActivationFunctionType.Copy,
                                         bias=0.0, scale=ALPHA)
                    # DVE order fills the idle window between bankB-ready
                    # and bankA-ready with the Tn/P2n/Dt work
                    nc.vector.tensor_add(En[:, :], bankB[:, :], tAB[:, :])
                    nc.vector.tensor_add(Tn[:, :], Cx[:, :], tTW[:, :])
                    nc.vector.tensor_add(Dt[:, :], Cx[:, :], Tn[:, :])
                    P2n = wpool.tile([P, CCOL], f32, tag="P2n")
                    # P2n = -(En + Tn): ready before bankA lands
                    nc.vector.scalar_tensor_tensor(P2n[:, :], En[:, :], -1.0,
                                                   Tn[:, :], Alu.mult, Alu.subtract)
                    nc.vector.tensor_add(At[:, :], bankA[:, :], En[:, :])
                    At_src = At[:, :]
                # soft threshold on DVE in 2 ops: soft(z) = z - clip(z,-lam,lam)
                nc.vector.tensor_scalar(r1[:, :], At_src, -LAM, LAM,
                                        Alu.max, Alu.min)
                nc.vector.tensor_sub(Un[:, :], At_src, r1[:, :])
                if first:
                    nc.vector.tensor_add(vm, Un[:, :], Wn[:, :])
                else:
                    P1 = wpool.tile([P, CCOL], f32, tag="P1")
                    # P1 = relu(Dt) + Un in one op; Wn state off-path (GpSimd)
                    nc.vector.scalar_tensor_tensor(P1[:, :], Dt[:, :], 0.0,
                                                   Un[:, :], Alu.max, Alu.add)
                    nc.gpsimd.tensor_scalar_max(Wn[:, :], Dt[:, :], 0.0)
                    nc.vector.tensor_add(vm, P1[:, :], P2n[:, :])
                halo_fix(vh)

                # y = G v + c0 (PE).  Emitted in two pieces with the
                # rank-9 reduce/broadcast matmul (bankR) in between, so the
                # Z2 chain starts while the G corner matmuls still run.

                # rank-9: q = U^T v via DVE; all-ones matmul reduces over
                # partitions AND broadcasts q to [128, 9]
                Z1 = wpool.tile([P, R9 * CCOL], f32, tag="Z1")
                Z1r = wpool.tile([P, R9], f32, tag="Z1r")
                z1_3d = Z1[:, :].rearrange("k (m c) -> k m c", c=CCOL)
                vb = vm.unsqueeze(1).broadcast_to([P, R9, CCOL])
                nc.vector.tensor_mul(z1_3d, U2_3d, vb)
                nc.vector.tensor_reduce(Z1r[:, :], z1_3d, axis=mybir.AxisListType.X,
                                        op=Alu.add)
                nc.tensor.matmul(bankE[:, :], I_t, c0_t, start=True, stop=False)
                nc.tensor.matmul(bankE[:, :], G_M, vm, start=False, stop=False)
                emit_bank([(bankR[:, :], onesf, Z1r[:, :])])
                nc.tensor.matmul(bankE[:, :], G_BL, vl, start=False, stop=False)
                nc.tensor.matmul(bankE[:, :], G_BH, vr, start=False, stop=True)

                Z2 = wpool.tile([P, CCOL * R9], f32, tag="Z2")
                corr = wpool.tile([P, CCOL], f32, tag="corr")
                z2_3d = Z2[:, :].rearrange("k (c m) -> k c m", m=R9)
                rb = bankR[:, :].unsqueeze(1).broadcast_to([P, CCOL, R9])
                nc.vector.tensor_mul(z2_3d, W2_3d, rb)
                nc.vector.tensor_reduce(corr[:, :], z2_3d, axis=mybir.AxisListType.X,
                                        op=Alu.add)

                # x' = bankE - corr   (c0 accumulated into bankE on PE)
                Xn = wpool.tile([P, HCOL], f32, tag="Xh")
                nc.vector.tensor_sub(Xn[:, 1:CCOL + 1], bankE[:, :], corr[:, :])
                halo_fix(Xn)

                Xh, E_sb, T_sb, U_sb, W_sb = Xn, En, Tn, Un, Wn

            nc.sync.dma_start(out_ext[:, :], Xh[:, 1:CCOL + 1])

    nc.compile()
    return nc


def kernel(**inputs):
    from concourse.bass_utils import run_bass_kernel_spmd

    target = np.asarray(inputs["target"], np.float32)
    A = np.asarray(inputs["A"], np.float32)
    x0 = np.asarray(inputs["x0"], np.float32)

    blob = host_constants(target, A, x0)
    nc = build_nc()
    in_maps = [{"blob": blob} for _ in range(NCORES)]
    res = run_bass_kernel_spmd(nc, in_maps, core_ids=list(range(NCORES)))
    out_tile = np.asarray(res.results[0]["out"], np.float32)
    return np.ascontiguousarray(out_tile.T.reshape(-1))
